# revision 10
# baseline (speedup 1.0000x reference)
"""Trainium2 Bass kernel for nn_BlipAttention_75007308857568.

Single-head BLIP attention: B=32, N=1024, C=768, fp32 reference.
  qkv = x @ qkv_w + qkv_b ; q,k,v split
  scores = q @ k.T / sqrt(C) ; attn = softmax(scores)
  out = attn @ v
  y = (out.swapaxes(1,2).reshape(B,N,C)) @ proj_w + proj_b

Sharding: data-parallel over batch B across 8 NeuronCores (4 batches/core).

Internal precision is bf16 (inputs converted host-side; fp32 PSUM
accumulation everywhere; measured rel err ~3e-3 vs the 2e-2 gate).
bf16 vs the fp32r predecessor: same 1 cycle/column matmul rate on the PE,
but transposes run 2x faster (1 vs 2 cyc/row), weight loads get FWL,
and DVE/DMA/SBUF traffic all halve.

Per-core dataflow (all in "transposed domain" to keep matmul contraction
dims on SBUF partitions without redundant transposes):
  XT  = x[b].T                      (48 PE transposes per batch, batched 4
                                     per PSUM bank + grouped copies)
  QT/KT = (Wq|Wk).T @ XT + bias     (PE, lhsT = weight chunk)
  V   = x[b] @ Wv + bias            (PE, lhsT = XT chunk -> natural layout)
  scoresT[m,n] = KT.T@QT            (PE, lhsT = KT chunk)
  expT = exp(scoresT/sqrt(C))       (ACT, PSUM->SBUF)
  denom = ones128.T @ expT          (PE; ones matrix -> denominator
                                     replicated on all 128 partitions)
  OT[c,n] = (V.T @ expT) * recip    (PE + DVE normalize, PSUM->SBUF)
  scratch flat = OT (c-major)       -> reinterpreting flat as [N,C] IS the
                                       swapaxes+reshape permutation for free
  PT = transpose(P rows)            (48 PE transposes per batch)
  y = P @ proj_w + proj_b           (PE, lhsT = PT chunk)
"""

import math
import os

import numpy as np
import ml_dtypes

import concourse.bacc as bacc
import concourse.bass as bass
import concourse.mybir as mybir
import concourse.tile as tile

_EW = None  # set in _build: nc.vector (default) or nc.any
from concourse.bass_utils import run_bass_kernel_spmd
from concourse.masks import make_identity

B, N, C = 32, 1024, 768
NCORES = 8
BPC = B // NCORES  # batches per core
CB = C // 128      # 6 channel blocks
NB = N // 128      # 8 sequence blocks
NH = 512           # n-half width (PSUM bank limit: 512 fp32 out cols)
SCALE = 1.0 / math.sqrt(C)

_CACHE = {}


def _build(mm_bf16: bool):
    dt = mybir.dt
    MM = dt.bfloat16 if mm_bf16 else dt.float32r
    f32 = dt.float32

    nc = bacc.Bacc("TRN2", target_bir_lowering=False, debug=False)
    global _EW
    _EW = nc.any if os.environ.get("BLIP_ANY") == "1" else nc.vector

    xs = nc.dram_tensor("xs", [BPC, N, C], MM, kind="ExternalInput")
    qkv_w = nc.dram_tensor("qkv_w", [C, 3 * C], MM, kind="ExternalInput")
    qkv_b = nc.dram_tensor("qkv_b", [3 * C], f32, kind="ExternalInput")
    proj_w = nc.dram_tensor("proj_w", [C, C], MM, kind="ExternalInput")
    proj_b = nc.dram_tensor("proj_b", [C], f32, kind="ExternalInput")
    y = nc.dram_tensor("y", [BPC, N, C], f32, kind="ExternalOutput")

    _wdma = nc.gpsimd if os.environ.get("BLIP_WDMA") == "gp" else nc.sync
    with tile.TileContext(nc) as tc:
        with (
            tc.tile_pool(name="consts", bufs=1) as consts,
            tc.tile_pool(name="xt", bufs=2) as pool_xt,
            tc.tile_pool(name="qt", bufs=1) as pool_qt,
            tc.tile_pool(name="kt", bufs=1) as pool_kt,
            tc.tile_pool(name="v", bufs=1) as pool_v,
            tc.tile_pool(name="expt", bufs=2) as pool_expt,
            tc.tile_pool(name="row", bufs=int(os.environ.get("BLIP_ROW", "3"))) as pool_row,
            tc.tile_pool(name="pt", bufs=int(os.environ.get("BLIP_PT", "4"))) as pool_pt,
            tc.tile_pool(name="rb", bufs=2) as pool_rb,
            tc.tile_pool(name="scr", bufs=int(os.environ.get("BLIP_SCR", "2")), space="DRAM") as pool_scr,
            tc.tile_pool(name="psmm", bufs=int(os.environ.get("BLIP_PSMM", "6")), space="PSUM") as psmm,
            tc.tile_pool(name="pst", bufs=int(os.environ.get("BLIP_PST", "2")), space="PSUM") as pst,
        ):
            # ---- constants / weights (loaded once) ----
            ident = consts.tile([128, 128], MM, tag="ident")
            make_identity(nc, ident)

            W = consts.tile([128, CB, 3 * C], MM, tag="W")
            PW = consts.tile([128, CB, C], MM, tag="PW")

            # q/k bias as per-partition scalars: qkb[p, ob] = qkv_b[ob*128+p]
            # Bias DMAs go on the (idle) ACT queue so batch-0 x-row loads on
            # the SP queue aren't stuck behind them at kernel start.
            qkb = consts.tile([128, 2 * CB], f32, tag="qkb")
            nc.scalar.dma_start(
                qkb, qkv_b.ap()[0 : 2 * C].rearrange("(ob p) -> p ob", p=128)
            )
            # v bias / proj bias replicated across partitions
            vb = consts.tile([128, C], f32, tag="vb")
            nc.scalar.dma_start(vb, qkv_b.ap()[None, 2 * C : 3 * C].to_broadcast([128, C]))
            pb = consts.tile([128, C], f32, tag="pb")
            nc.scalar.dma_start(pb, proj_b.ap()[None, :].to_broadcast([128, C]))

            # ones matrix: denominator matmul replicates the column sums of
            # expT onto all 128 output partitions.
            ones = consts.tile([128, 128], MM, tag="ones")
            nc.vector.memset(ones, 1.0)

            tcopy_act = os.environ.get("BLIP_TCOPY") == "act"

            def _tcopy(out, in_):
                if tcopy_act:
                    nc.scalar.copy(out, in_)
                else:
                    _EW.tensor_copy(out, in_)

            def transpose_block(src_row, dst, dst_slices):
                """PE-transpose six 128x128 chunks of src_row, batched 4+2
                per PSUM bank, with one grouped copy per bank."""
                psA = pst.tile([128, NH], MM, tag="tp4")
                for k in range(4):
                    nc.tensor.transpose(
                        psA[:, k * 128 : (k + 1) * 128],
                        src_row[:, k * 128 : (k + 1) * 128],
                        ident,
                    )
                _tcopy(
                    dst_slices[0], psA.rearrange("p (c k) -> p c k", k=128)
                )
                psB = pst.tile([128, NH], MM, tag="tp4")
                for k in range(2):
                    nc.tensor.transpose(
                        psB[:, k * 128 : (k + 1) * 128],
                        src_row[:, (4 + k) * 128 : (5 + k) * 128],
                        ident,
                    )
                _tcopy(
                    dst_slices[1],
                    psB[:, 0:256].rearrange("p (c k) -> p c k", k=128),
                )

            def stage_a(b):
                """XT = x[b].T"""
                XT = pool_xt.tile([128, CB, N], MM, tag="XT")
                for nb in range(NB):
                    xrow = pool_row.tile([128, C], MM, tag="row")
                    nc.sync.dma_start(xrow, xs.ap()[b, nb * 128 : (nb + 1) * 128, :])
                    nsl = slice(nb * 128, (nb + 1) * 128)
                    transpose_block(
                        xrow, XT, [XT[:, 0:4, nsl], XT[:, 4:6, nsl]]
                    )
                return XT

            # batch-0 x rows load before the big weight DMAs so the PE can
            # start transposing immediately; weights stream chunk-wise in the
            # same order stage B consumes them.
            import contextlib
            _loop_n = int(os.environ.get("BLIP_LOOP", "0"))
            _loop_ctx = tc.For_i(0, _loop_n, 1) if _loop_n else contextlib.nullcontext()
            _loop_ctx.__enter__()
            XT_next = stage_a(0)
            w_view = qkv_w.rearrange("(cb p) o -> p cb o", p=128)
            pw_view = proj_w.rearrange("(cb p) o -> p cb o", p=128)
            # weights stream on the ACT queue, in parallel with the x-row
            # loads on the SP queue that feed the batch-0 transposes.
            for cb in range(CB):
                nc.scalar.dma_start(W[:, cb], w_view[:, cb])
            for cb in range(CB):
                nc.scalar.dma_start(PW[:, cb], pw_view[:, cb])

            def stage_b(XT):
                """QT, KT (transposed domain), V (natural)"""
                QT = pool_qt.tile([128, CB, N], MM, tag="QT")
                KT = pool_kt.tile([128, CB, N], MM, tag="KT")
                for ob in range(2 * CB):
                    dest = QT if ob < CB else KT
                    dcb = ob % CB
                    for nh in range(N // NH):
                        ps = psmm.tile([128, NH], f32, tag="mm")
                        for cb in range(CB):
                            nc.tensor.matmul(
                                ps,
                                W[:, cb, ob * 128 : (ob + 1) * 128],
                                XT[:, cb, nh * NH : (nh + 1) * NH],
                                start=(cb == 0),
                                stop=(cb == CB - 1),
                            )
                        _EW.tensor_scalar(
                            dest[:, dcb, nh * NH : (nh + 1) * NH],
                            ps,
                            qkb[:, ob : ob + 1],
                            None,
                            op0=mybir.AluOpType.add,
                        )

                V = pool_v.tile([128, NB, C], MM, tag="V")
                for mb in range(NB):
                    for c0, cw in ((0, NH), (NH, C - NH)):
                        ps = psmm.tile([128, NH], f32, tag="mm")
                        for cb in range(CB):
                            nc.tensor.matmul(
                                ps[:, :cw],
                                XT[:, cb, mb * 128 : (mb + 1) * 128],
                                W[:, cb, 2 * C + c0 : 2 * C + c0 + cw],
                                start=(cb == 0),
                                stop=(cb == CB - 1),
                            )
                        _EW.tensor_tensor(
                            V[:, mb, c0 : c0 + cw],
                            ps[:, :cw],
                            vb[:, c0 : c0 + cw],
                            op=mybir.AluOpType.add,
                        )
                return QT, KT, V

            def stage_cd(QT, KT, V):
                """attention; returns scr holding OT flat.

                Both n-halves' expT (+ reciprocal denominators) are computed
                first, then attn@V runs channel-major over both halves so the
                scratch rows complete in the order stage_e consumes them."""
                scr = pool_scr.tile([C * N], MM, tag="scr")
                scrv = scr.rearrange("(c n) -> c n", n=N)
                expTs, recips = [], []
                for nh in range(N // NH):
                    nsl = slice(nh * NH, (nh + 1) * NH)
                    expT = pool_expt.tile([128, NB, NH], MM, tag="expT")
                    for mb in range(NB):
                        ps = psmm.tile([128, NH], f32, tag="mm")
                        for cb in range(CB):
                            nc.tensor.matmul(
                                ps,
                                KT[:, cb, mb * 128 : (mb + 1) * 128],
                                QT[:, cb, nsl],
                                start=(cb == 0),
                                stop=(cb == CB - 1),
                            )
                        nc.scalar.activation(
                            expT[:, mb, :], ps, mybir.ActivationFunctionType.Exp,
                            scale=SCALE,
                        )

                    dps = psmm.tile([128, NH], f32, tag="mm")
                    for mb in range(NB):
                        nc.tensor.matmul(
                            dps, ones, expT[:, mb, :],
                            start=(mb == 0), stop=(mb == NB - 1),
                        )
                    recipB = pool_rb.tile([128, NH], f32, tag="recipB")
                    nc.vector.reciprocal(recipB, dps)
                    expTs.append(expT)
                    recips.append(recipB)

                for cb in range(CB):
                    for nh in range(N // NH):
                        nsl = slice(nh * NH, (nh + 1) * NH)
                        ps = psmm.tile([128, NH], f32, tag="mm")
                        for mb in range(NB):
                            nc.tensor.matmul(
                                ps,
                                V[:, mb, cb * 128 : (cb + 1) * 128],
                                expTs[nh][:, mb, :],
                                start=(mb == 0),
                                stop=(mb == NB - 1),
                            )
                        ot = pool_row.tile([128, NH], MM, tag="row")
                        _EW.tensor_tensor(ot, ps, recips[nh], op=mybir.AluOpType.mult)
                        _wdma.dma_start(scrv[cb * 128 : (cb + 1) * 128, nsl], ot)
                return scr

            def stage_e(scr, b):
                """y = P @ proj_w + proj_b.
                P = flat(OT) viewed as [N, C]; the swapaxes+reshape for free.

                The prow load + PE transpose + PSUM->SBUF copy for row ib+1
                are emitted BEFORE row ib's projection matmuls: the DVE queue
                is FIFO, so the pt copies (which gate the next matmul group)
                must enqueue ahead of the non-critical yrow bias adds."""
                pview = scr.rearrange("(i j) -> i j", j=C)

                def load_transpose(ib):
                    prow = pool_row.tile([128, C], MM, tag="row")
                    nc.sync.dma_start(prow, pview[ib * 128 : (ib + 1) * 128, :])
                    pt4a = pool_pt.tile([128, NH], MM, tag="pt4")
                    pt4b = pool_pt.tile([128, NH], MM, tag="pt4")
                    transpose_block(
                        prow,
                        None,
                        [
                            pt4a.rearrange("p (c k) -> p c k", k=128),
                            pt4b[:, 0:256].rearrange("p (c k) -> p c k", k=128),
                        ],
                    )
                    return pt4a, pt4b

                pts = load_transpose(0)
                for ib in range(NB):
                    pt4a, pt4b = pts
                    if ib + 1 < NB:
                        pts = load_transpose(ib + 1)
                    ps1 = psmm.tile([128, NH], f32, tag="mm")
                    ps2 = psmm.tile([128, NH], f32, tag="mm")
                    for jb in range(CB):
                        pt = (pt4a if jb < 4 else pt4b)[
                            :, (jb % 4) * 128 : (jb % 4 + 1) * 128
                        ]
                        nc.tensor.matmul(
                            ps1, pt, PW[:, jb, 0:NH],
                            start=(jb == 0), stop=(jb == CB - 1),
                        )
                        nc.tensor.matmul(
                            ps2[:, : C - NH], pt, PW[:, jb, NH:C],
                            start=(jb == 0), stop=(jb == CB - 1),
                        )
                    yrow = pool_row.tile([128, C], f32, tag="yrow")
                    _EW.tensor_tensor(
                        yrow[:, 0:NH], ps1, pb[:, 0:NH], op=mybir.AluOpType.add
                    )
                    _EW.tensor_tensor(
                        yrow[:, NH:C], ps2[:, : C - NH], pb[:, NH:C],
                        op=mybir.AluOpType.add,
                    )
                    _wdma.dma_start(y.ap()[b, ib * 128 : (ib + 1) * 128, :], yrow)

            # Software pipeline across batches: next batch's transposes and
            # QKV matmuls are emitted before this batch's projection stage so
            # the scheduler can fill stage-E's DMA-bound stretch with PE work.
            qkv = stage_b(XT_next)
            for b in range(BPC):
                if b + 1 < BPC:
                    XT = stage_a(b + 1)  # x loads go out during quiet phase C
                scr = stage_cd(*qkv)
                if b + 1 < BPC:
                    qkv = stage_b(XT)
                stage_e(scr, b)
            _loop_ctx.__exit__(None, None, None)

    nc.compile()
    return nc


def _get_nc():
    mm_bf16 = os.environ.get("BLIP_MM_DTYPE", "bfloat16") != "float32r"
    key = ("nc", mm_bf16)
    if key not in _CACHE:
        _CACHE[key] = _build(mm_bf16)
    return _CACHE[key]


def _mm_np_dtype():
    mm_bf16 = os.environ.get("BLIP_MM_DTYPE", "bfloat16") != "float32r"
    return ml_dtypes.bfloat16 if mm_bf16 else np.float32


def _make_in_maps(inputs):
    mdt = _mm_np_dtype()
    x = np.ascontiguousarray(np.asarray(inputs["x"]).astype(mdt))
    shared = {
        "qkv_w": np.ascontiguousarray(np.asarray(inputs["qkv_w"]).astype(mdt)),
        "qkv_b": np.ascontiguousarray(np.asarray(inputs["qkv_b"], dtype=np.float32)),
        "proj_w": np.ascontiguousarray(np.asarray(inputs["proj_w"]).astype(mdt)),
        "proj_b": np.ascontiguousarray(np.asarray(inputs["proj_b"], dtype=np.float32)),
    }
    return [
        {"xs": x[c * BPC : (c + 1) * BPC], **shared} for c in range(NCORES)
    ]


def kernel(x, qkv_w, qkv_b, proj_w, proj_b, _trace=False, _tmpdir=None):
    nc = _get_nc()
    in_maps = _make_in_maps(
        {"x": x, "qkv_w": qkv_w, "qkv_b": qkv_b, "proj_w": proj_w, "proj_b": proj_b}
    )
    res = run_bass_kernel_spmd(
        nc, in_maps, core_ids=list(range(NCORES)),
        trace=_trace, tmpdir=_tmpdir,
        **({"trace_cores": [0]} if _trace else {}),
    )
    out = np.concatenate([res.results[c]["y"] for c in range(NCORES)], axis=0)
    if _trace:
        return out, res
    return out
